# revision 1
# baseline (speedup 1.0000x reference)
"""Block-local multi-head attention (nn_MultiHeadFlashAttention) on 8 TRN2 cores.

Sharding: the computation is fully independent per 128-token block
(qkv/proj are per-token, attention is block-local), so we split the
B*T = 16384 tokens into 8 contiguous shards of 2048 tokens (half a batch
element each). No collectives needed.

Per-core kernel (tokens = 2048, processed in 4 groups of 512):
  - x^T shard [C=1024, 2048] is pre-transposed + bf16-cast on host so all
    DMAs are contiguous; it stays fully SBUF-resident (32 KB/partition).
  - qkv projection with weights stationary: q^T,k^T = W^T x^T tiles
    ([feat, tok] layout) and v = x W_v ([tok, feat] layout).
  - Per 128-block, per head: scores = q_h k_h^T via PE (K=64), causal-mask
    + scale fused in one DVE scalar_tensor_tensor reading PSUM, exp with
    fused row-sum on ACT, normalize via tensor_scalar, p^T via PE
    transpose, attn^T = v_h^T p^T via PE.
  - proj: out = attn^T.T W_proj + b_proj (bias folded in as a K=1
    outer-product matmul into the same PSUM accumulation).

DMA discipline: this toolchain's walrus only accepts ONE sync wait per
DMA instruction. So all loads are issued once, upfront, on the gpsimd SW
queue (no data deps -> at most a ring wait), and there are exactly 8
stores (two blocks each) on the SP HW queue's 8 rings (fresh ring each ->
only the RAW wait on the producer).

Numerics: bf16 matmul operands, fp32 PSUM accumulation and softmax
intermediates. Max-subtraction is skipped (scores are O(1) bounded);
masked logits get -1e5 so exp underflows to exactly 0.
"""

import numpy as np
import ml_dtypes
from contextlib import ExitStack

import concourse.bass as bass
import concourse.bacc as bacc
import concourse.mybir as mybir
import concourse.tile as tile
from concourse.masks import make_identity
from concourse import bass_utils

BF16 = mybir.dt.bfloat16
F32 = mybir.dt.float32

B, T, C = 4, 4096, 1024
H, D, BS = 16, 64, 128
N_CORES = 8
TOK = (B * T) // N_CORES        # 2048 tokens per core
GTOK = 512                      # tokens per group (matmul moving dim)
NG = TOK // GTOK                # 4 groups
GB = GTOK // BS                 # 4 blocks per group
KT = C // 128                   # 8 contraction tiles
SCALE = 1.0 / np.sqrt(D)
MASK_VAL = -1.0e5
REPEAT = 1          # timing builds only: repeat the whole body in-kernel

_CACHE = {}


def _build_body(nc, tc, ctx, xT, w_qkv, w_proj, b_proj, out):
    # ---- resident tiles, loaded upfront on the PL (gpsimd) queue.
    # Split so the first qkv matmuls can start as soon as the q-weights and
    # group-0 x^T land (startup latency), and ordered by first use. ----
    const = ctx.enter_context(tc.tile_pool(name="const", bufs=1))
    wq_r = w_qkv.rearrange("(kt p) f -> p kt f", p=128)
    xT_r = xT.rearrange("(kt p) t -> p kt t", p=128)
    wq_q = const.tile([128, KT, C], BF16, tag="wq_q")   # 16 KB/part each
    wq_k = const.tile([128, KT, C], BF16, tag="wq_k")
    wq_v = const.tile([128, KT, C], BF16, tag="wq_v")
    xts = []
    nc.gpsimd.dma_start(wq_q[:], wq_r[:, :, 0:C])
    for g in range(NG):
        t = const.tile([128, KT, GTOK], BF16, tag=f"xt{g}")
        nc.gpsimd.dma_start(t[:], xT_r[:, :, g * GTOK:(g + 1) * GTOK])
        xts.append(t)
        if g == 0:
            nc.gpsimd.dma_start(wq_k[:], wq_r[:, :, C:2 * C])
            nc.gpsimd.dma_start(wq_v[:], wq_r[:, :, 2 * C:3 * C])
    wp_sb = const.tile([128, KT, C], BF16)           # 16 KB/part
    nc.gpsimd.dma_start(wp_sb[:], w_proj.rearrange("(kt p) f -> p kt f", p=128))
    bias_sb = const.tile([1, C], BF16)
    nc.gpsimd.dma_start(bias_sb[:], b_proj[:])

    ones_sb = const.tile([1, 128], BF16)
    nc.vector.memset(ones_sb[:], 1.0)
    ident = const.tile([128, 128], BF16)
    make_identity(nc, ident[:])
    # causal 0/1 mask (lower triangular): applied AFTER exp by multiply
    tril = const.tile([128, BS], BF16)
    nc.gpsimd.memset(tril[:], 1.0)
    nc.gpsimd.affine_select(
        out=tril[:], in_=tril[:],
        compare_op=mybir.AluOpType.is_ge,
        fill=0.0, base=0,
        pattern=[[-1, BS]],  # iota = q - k, keep 1.0 where >= 0
        channel_multiplier=1,
    )

    # ---- working pools (SBUF) ----
    qk_pool = ctx.enter_context(tc.tile_pool(name="qk", bufs=2))
    v_pool = ctx.enter_context(tc.tile_pool(name="v", bufs=1))
    e_pool = ctx.enter_context(tc.tile_pool(name="e", bufs=2))
    p_pool = ctx.enter_context(tc.tile_pool(name="p", bufs=2))
    den_pool = ctx.enter_context(tc.tile_pool(name="den", bufs=2))
    pt_pool = ctx.enter_context(tc.tile_pool(name="pt", bufs=3))
    at_pool = ctx.enter_context(tc.tile_pool(name="at", bufs=2))
    out_pool = ctx.enter_context(tc.tile_pool(name="out", bufs=2))

    # ---- PSUM pools (8 banks total) ----
    qkv_ps = ctx.enter_context(tc.tile_pool(name="qkv_ps", bufs=2, space="PSUM"))
    proj_ps = ctx.enter_context(tc.tile_pool(name="proj_ps", bufs=1, space="PSUM"))
    sc_ps = ctx.enter_context(tc.tile_pool(name="sc_ps", bufs=1, space="PSUM"))
    pt_ps = ctx.enter_context(tc.tile_pool(name="pt_ps", bufs=2, space="PSUM"))
    at_ps = ctx.enter_context(tc.tile_pool(name="at_ps", bufs=2, space="PSUM"))

    for rep in range(REPEAT):
      for g in range(NG):
        t0 = g * GTOK
        xt = xts[g]

        # q^T, k^T: feature-major [feat_tile 128, ft 16, tok 512]
        qkT = qk_pool.tile([128, 2 * H, GTOK], BF16)
        for ft in range(16):  # 8 q tiles then 8 k tiles
            ps = qkv_ps.tile([128, GTOK], F32, tag="mm512")
            for kt in range(KT):
                w = wq_q if ft < 8 else wq_k
                fo = (ft % 8) * 128
                nc.tensor.matmul(
                    ps[:],
                    lhsT=w[:, kt, fo:fo + 128],
                    rhs=xt[:, kt, :],
                    start=(kt == 0), stop=(kt == KT - 1),
                )
            nc.scalar.copy(qkT[:, ft, :], ps[:])

        # v: token-major [tok 128, tt 4, feat 1024]
        v = v_pool.tile([128, GB, C], BF16)
        for tt in range(GB):
            for ns in range(2):
                ps = qkv_ps.tile([128, GTOK], F32, tag="mm512")
                for kt in range(KT):
                    nc.tensor.matmul(
                        ps[:],
                        lhsT=xt[:, kt, tt * 128:(tt + 1) * 128],
                        rhs=wq_v[:, kt, ns * 512:(ns + 1) * 512],
                        start=(kt == 0), stop=(kt == KT - 1),
                    )
                nc.scalar.copy(v[:, tt, ns * 512:(ns + 1) * 512], ps[:])

        for bp in range(GB // 2):   # block pairs -> one store DMA each
            ob = out_pool.tile([128, 2, C], F32)
            for bj in range(2):
                b = bp * 2 + bj
                tok = slice(b * BS, (b + 1) * BS)
                p_sb = p_pool.tile([128, H, BS], BF16, tag="p")
                den = den_pool.tile([128, H], F32, tag="den")
                # Heads grouped by q/k partition parity: matmuls sharing a
                # PSUM bank must come from the same PE row-group (mixed
                # row-group writes to one bank fault the hardware).
                for half in (0, 2, 1, 3):
                    parity, bft = half // 2, (half % 2) * 4
                    sps = sc_ps.tile([128, 4, BS], F32)
                    for hh in range(4):
                        ft, po = bft + hh, 64 * parity
                        nc.tensor.matmul(
                            sps[:, hh, :],
                            lhsT=qkT[po:po + 64, ft, tok],
                            rhs=qkT[po:po + 64, 8 + ft, tok],
                            start=True, stop=True,
                        )
                    # e = exp(scores * scale), unmasked (scores are O(1))
                    e_sb = e_pool.tile([128, 4, BS], BF16, tag="e")
                    nc.scalar.activation(
                        e_sb[:], sps[:], mybir.ActivationFunctionType.Exp,
                        scale=float(SCALE),
                    )
                    # p_unnorm = e * tril01, with fused row-sum -> den
                    for hh in range(4):
                        h = 2 * (bft + hh) + parity
                        slot = half * 4 + hh
                        nc.vector.scalar_tensor_tensor(
                            out=p_sb[:, h, :], in0=e_sb[:, hh, :], scalar=1.0,
                            in1=tril[:],
                            op0=mybir.AluOpType.mult, op1=mybir.AluOpType.mult,
                            accum_out=den[:, slot:slot + 1],
                        )
                    dsl = slice(half * 4, (half + 1) * 4)
                    nc.vector.tensor_scalar_add(den[:, dsl], den[:, dsl], 1e-6)
                    nc.vector.reciprocal(den[:, dsl], den[:, dsl])

                atn = at_pool.tile([128, KT, BS], BF16, tag="at")
                for h in range(H):
                    ft, parity = h // 2, h % 2
                    slot = (parity * 2 + ft // 4) * 4 + ft % 4
                    nc.vector.tensor_scalar_mul(
                        p_sb[:, h, :], p_sb[:, h, :], den[:, slot:slot + 1])
                    ptp = pt_ps.tile([128, BS], BF16)
                    nc.tensor.transpose(ptp[:], p_sb[:, h, :], ident[:])
                    pt = pt_pool.tile([128, BS], BF16)
                    nc.vector.tensor_copy(pt[:], ptp[:])
                    po = 64 * (h % 2)
                    if h % 2 == 0:
                        atp = at_ps.tile([128, BS], F32)
                    nc.tensor.matmul(
                        atp[po:po + 64, :],
                        lhsT=v[:, b, h * D:(h + 1) * D],
                        rhs=pt[:],
                        start=True, stop=True,
                        tile_position=(0, po),
                    )
                    if h % 2 == 1:
                        nc.scalar.copy(atn[:, h // 2, :], atp[:])

                # proj: out[tok, cout] = attn^T.T @ W_proj + b_proj
                for ns in range(2):
                    pps = proj_ps.tile([128, 512], F32)
                    for ct in range(KT):
                        nc.tensor.matmul(
                            pps[:],
                            lhsT=atn[:, ct, :],
                            rhs=wp_sb[:, ct, ns * 512:(ns + 1) * 512],
                            start=(ct == 0), stop=False,
                        )
                    nc.tensor.matmul(
                        pps[:],
                        lhsT=ones_sb[:1, :],
                        rhs=bias_sb[:1, ns * 512:(ns + 1) * 512],
                        start=False, stop=True,
                    )
                    nc.scalar.copy(ob[:, bj, ns * 512:(ns + 1) * 512], pps[:])

            # one store per block pair on the SP HW queue (8 total).
            # Timing builds (REPEAT>1) store only on the last repeat so
            # every store DMA still carries a single sync wait.
            if rep == REPEAT - 1:
                r0 = t0 + bp * 2 * BS
                nc.sync.dma_start(
                    out[r0:r0 + 2 * BS, :].rearrange("(blk p) c -> p blk c", p=128),
                    ob[:],
                )


def _build():
    nc = bacc.Bacc()
    xT = nc.dram_tensor("xT", [C, TOK], BF16, kind="ExternalInput")
    w_qkv = nc.dram_tensor("w_qkv", [C, 3 * C], BF16, kind="ExternalInput")
    w_proj = nc.dram_tensor("w_proj", [C, C], BF16, kind="ExternalInput")
    b_proj = nc.dram_tensor("b_proj", [1, C], BF16, kind="ExternalInput")
    out = nc.dram_tensor("out", [TOK, C], F32, kind="ExternalOutput")
    with tile.TileContext(nc) as tc:
        with ExitStack() as ctx:
            _build_body(nc, tc, ctx, xT, w_qkv, w_proj, b_proj, out)
    nc.finalize()
    return nc


def get_nc():
    key = f"nc{REPEAT}"
    if key not in _CACHE:
        _CACHE[key] = _build()
    return _CACHE[key]


def make_in_maps(x, W_qkv, W_proj, b_proj):
    bf = ml_dtypes.bfloat16
    wq = np.ascontiguousarray(W_qkv.astype(bf))
    wp = np.ascontiguousarray(W_proj.astype(bf))
    bp = np.ascontiguousarray(b_proj.reshape(1, C).astype(bf))
    xb = x.astype(bf)
    in_maps = []
    for s in range(N_CORES):
        bi, half = divmod(s, 2)
        xs = xb[bi, half * TOK:(half + 1) * TOK]      # [2048, 1024]
        in_maps.append({
            "xT": np.ascontiguousarray(xs.T),
            "w_qkv": wq, "w_proj": wp, "b_proj": bp,
        })
    return in_maps


def kernel(x, W_qkv, W_proj, b_proj, _trace=False):
    nc = get_nc()
    in_maps = make_in_maps(x, W_qkv, W_proj, b_proj)
    res = bass_utils.run_bass_kernel_spmd(
        nc, in_maps, core_ids=list(range(N_CORES)), trace=_trace,
    )
    _CACHE["last_result"] = res
    out = np.empty((B, T, C), np.float32)
    for s in range(N_CORES):
        bi, half = divmod(s, 2)
        out[bi, half * TOK:(half + 1) * TOK] = res.results[s]["out"]
    return out



# revision 5
# speedup vs baseline: 1.1300x; 1.1300x over previous
"""Block-local multi-head attention (nn_MultiHeadFlashAttention) on 8 TRN2 cores.

Sharding: fully independent per 128-token block (qkv/proj are per-token,
attention is block-local), so the B*T = 16384 tokens split into 8 contiguous
shards of 2048 tokens. No collectives.

Per-core kernel (tokens = 2048, processed in 4 groups of 512):
  - q,k projection in fp8-e4m3 DoubleRow (2 k-tiles per PE instruction):
    softmax smooths q/k quantization error, so single fp8 is inside the
    rel-err budget. Weights are pre-scaled x256 on host so they sit in
    e4m3's normal range; the 65536x score scale folds into the exp scale.
  - v projection in fp8 DoubleRow with hi+lo residual compensation
    (x = x_hi + x_lo, 256*Wv = Wv_hi + Wv_lo, three cross terms, lo*lo
    dropped) -> bf16-level accuracy at 0.75x the bf16 PE cost. The v-path
    error passes through p@v un-smoothed, so plain fp8 would fail there.
  - attention per 128-block: scores on PE (64-row head-parity grouping per
    PSUM bank), exp on ACT, causal mask via one gpsimd affine_select over
    all 16 heads, row-sums on DVE tensor_reduce, p = e/den via one gpsimd
    tensor-tensor divide with a 0-stride broadcast AP, p^T via PE transpose
    (4 heads per PSUM bank -> single copy), attn^T = v_h^T p^T on PE packed
    2 heads per bank column-group.
  - proj in bf16 (fp8 fails numerics there), bias folded in as a K=1
    outer-product matmul, PSUM->SBUF copies balanced across ACT and DVE.

DMA discipline: one sync wait per DMA instruction. All loads are issued
once, upfront, on the gpsimd SW queue in first-use order; exactly 8 stores
(two blocks each) go on the SP HW queue's 8 rings.

Numerics: fp8/bf16 matmul operands, fp32 PSUM and softmax intermediates.
Max-subtraction is skipped (scores are O(1) bounded); the causal mask zeroes
e post-exp, so masked lanes contribute exactly 0 to the row sums.
"""

import numpy as np
import ml_dtypes
from contextlib import ExitStack

import concourse.bass as bass
import concourse.bacc as bacc
import concourse.mybir as mybir
import concourse.tile as tile
from concourse.masks import make_identity
from concourse import bass_utils

BF16 = mybir.dt.bfloat16
F32 = mybir.dt.float32
F8 = mybir.dt.float8e4

B, T, C = 4, 4096, 1024
H, D, BS = 16, 64, 128
N_CORES = 8
TOK = (B * T) // N_CORES        # 2048 tokens per core
GTOK = 512                      # tokens per group
NG = TOK // GTOK                # 4 groups
GB = GTOK // BS                 # 4 blocks per group
KT = C // 128                   # 8 contraction tiles (4 DoubleRow pairs)
W_SCALE = 256.0                 # host pre-scale on W_qkv/W_v for e4m3 range
EXP_SCALE = 1.0 / (np.sqrt(D) * W_SCALE * W_SCALE)
AT_SCALE = 1.0 / W_SCALE        # de-scale attn output at the PSUM copy

# slot ordering within a block: quads of heads sharing q/k partition parity
# (matmuls sharing a PSUM bank must come from the same PE row-group).
# half in (0,2,1,3): parity = half//2, head = 2*(4*(half%2)+hh) + parity
HALves = (0, 2, 1, 3)
SLOT_HEADS = []
for _half in HALves:
    _par, _bft = _half // 2, (_half % 2) * 4
    for _hh in range(4):
        SLOT_HEADS.append(2 * (_bft + _hh) + _par)
SLOT_OF_HEAD = {h: s for s, h in enumerate(SLOT_HEADS)}

_CACHE = {}


def _bcast_last(ap_small, ap_big):
    """0-stride broadcast of [P, H, 1] onto [P, H, N]."""
    a, b = bass.broadcast_tensor_aps(ap_big, ap_small)
    return b


def _build_body(nc, tc, ctx, xhi, xlo, wqk, wvh, wvl, wp, bias, out):
    DR = mybir.MatmulPerfMode.DoubleRow

    # ---- resident tiles, loaded upfront on the PL (gpsimd) queue,
    # ordered by first use ----
    const = ctx.enter_context(tc.tile_pool(name="const", bufs=1))
    wqk_r = wqk.rearrange("(kt p) f -> p kt f", p=128)
    wvh_r = wvh.rearrange("(kt p) f -> p kt f", p=128)
    wvl_r = wvl.rearrange("(kt p) f -> p kt f", p=128)
    xhi_r = xhi.rearrange("(kt p) t -> p kt t", p=128)
    xlo_r = xlo.rearrange("(kt p) t -> p kt t", p=128)

    wqk_sb = const.tile([128, KT, 2 * C], F8)    # 16 KB/part
    xhi_sb = const.tile([128, KT, TOK], F8)      # 16 KB/part
    xlo_sb = const.tile([128, KT, TOK], F8)      # 16 KB/part
    wvh_sb = const.tile([128, KT, C], F8)        # 8 KB/part
    wvl_sb = const.tile([128, KT, C], F8)        # 8 KB/part
    wp_sb = const.tile([128, KT, C], BF16)       # 16 KB/part

    nc.gpsimd.dma_start(wqk_sb[:, :, 0:C], wqk_r[:, :, 0:C])
    nc.gpsimd.dma_start(xhi_sb[:, :, 0:GTOK], xhi_r[:, :, 0:GTOK])
    nc.gpsimd.dma_start(wqk_sb[:, :, C:2 * C], wqk_r[:, :, C:2 * C])
    nc.gpsimd.dma_start(wvh_sb[:], wvh_r[:])
    nc.gpsimd.dma_start(xlo_sb[:, :, 0:GTOK], xlo_r[:, :, 0:GTOK])
    nc.gpsimd.dma_start(wvl_sb[:], wvl_r[:])
    for g in range(1, NG):
        sl = slice(g * GTOK, (g + 1) * GTOK)
        nc.gpsimd.dma_start(xhi_sb[:, :, sl], xhi_r[:, :, sl])
        nc.gpsimd.dma_start(xlo_sb[:, :, sl], xlo_r[:, :, sl])
    nc.gpsimd.dma_start(wp_sb[:], wp.rearrange("(kt p) f -> p kt f", p=128))
    bias_sb = const.tile([1, C], BF16)
    nc.gpsimd.dma_start(bias_sb[:], bias[:])

    ones_sb = const.tile([1, 128], BF16)
    nc.vector.memset(ones_sb[:], 1.0)
    ident = const.tile([128, 128], BF16)
    make_identity(nc, ident[:])

    # ---- working pools (SBUF) ----
    qk_pool = ctx.enter_context(tc.tile_pool(name="qk", bufs=2))
    v_pool = ctx.enter_context(tc.tile_pool(name="v", bufs=2))
    e_pool = ctx.enter_context(tc.tile_pool(name="e", bufs=2))
    p_pool = ctx.enter_context(tc.tile_pool(name="p", bufs=2))
    den_pool = ctx.enter_context(tc.tile_pool(name="den", bufs=2))
    pt_pool = ctx.enter_context(tc.tile_pool(name="pt", bufs=3))
    at_pool = ctx.enter_context(tc.tile_pool(name="at", bufs=2))
    out_pool = ctx.enter_context(tc.tile_pool(name="out", bufs=2))

    # ---- PSUM pools (16 KB/partition total; this uses 14) ----
    mm_ps = ctx.enter_context(tc.tile_pool(name="mm_ps", bufs=2, space="PSUM"))
    sc_ps = ctx.enter_context(tc.tile_pool(name="sc_ps", bufs=2, space="PSUM"))
    pt_ps = ctx.enter_context(tc.tile_pool(name="pt_ps", bufs=2, space="PSUM"))
    at_ps = ctx.enter_context(tc.tile_pool(name="at_ps", bufs=2, space="PSUM"))

    for g in range(NG):
        t0 = g * GTOK
        gsl = slice(t0, t0 + GTOK)

        # ---- q^T, k^T: feature-major [feat 128, ft 16, tok 512], fp8 DR ----
        qkT = qk_pool.tile([128, 2 * H, GTOK], BF16, tag="qkT")
        for ft in range(16):
            ps = mm_ps.tile([128, GTOK], F32, tag="mm512")
            fo = ft * 128
            for pr in range(KT // 2):
                nc.tensor.matmul(
                    ps[:],
                    lhsT=wqk_sb[:, 2 * pr:2 * pr + 2, fo:fo + 128],
                    rhs=xhi_sb[:, 2 * pr:2 * pr + 2, gsl],
                    start=(pr == 0), stop=(pr == KT // 2 - 1),
                    perf_mode=DR,
                )
            if ft % 2 == 0:
                nc.scalar.copy(qkT[:, ft, :], ps[:])
            else:
                nc.vector.tensor_copy(qkT[:, ft, :], ps[:])

        # ---- v: token-major [tok 128, tt 4, feat 1024], x256 scale,
        # fp8 DR with hi/lo residual compensation (3 of 4 cross terms) ----
        v = v_pool.tile([128, GB, C], BF16, tag="v")
        for tt in range(GB):
            tsl = slice(t0 + tt * BS, t0 + (tt + 1) * BS)
            for ns in range(2):
                ps = mm_ps.tile([128, GTOK], F32, tag="mm512")
                nsl = slice(ns * 512, (ns + 1) * 512)
                n_mm = 3 * (KT // 2)
                i = 0
                for lhs_x, rhs_w in ((xhi_sb, wvh_sb), (xlo_sb, wvh_sb),
                                     (xhi_sb, wvl_sb)):
                    for pr in range(KT // 2):
                        nc.tensor.matmul(
                            ps[:],
                            lhsT=lhs_x[:, 2 * pr:2 * pr + 2, tsl],
                            rhs=rhs_w[:, 2 * pr:2 * pr + 2, nsl],
                            start=(i == 0), stop=(i == n_mm - 1),
                            perf_mode=DR,
                        )
                        i += 1
                if (tt * 2 + ns) % 2 == 0:
                    nc.scalar.copy(v[:, tt, nsl], ps[:])
                else:
                    nc.vector.tensor_copy(v[:, tt, nsl], ps[:])

        # ---- attention + proj per 128-block ----
        for bp in range(GB // 2):   # block pairs -> one store DMA each
            ob = out_pool.tile([128, 2, C], F32, tag="ob")
            for bj in range(2):
                b = bp * 2 + bj
                tok = slice(b * BS, (b + 1) * BS)

                # scores -> exp, 4 heads (one parity) per PSUM bank
                e_sb = e_pool.tile([128, H, BS], BF16, tag="e")
                for q4, half in enumerate(HALves):
                    parity, bft = half // 2, (half % 2) * 4
                    po = 64 * parity
                    sps = sc_ps.tile([128, 4, BS], F32, tag="sc")
                    for hh in range(4):
                        ft = bft + hh
                        nc.tensor.matmul(
                            sps[:, hh, :],
                            lhsT=qkT[po:po + 64, ft, tok],
                            rhs=qkT[po:po + 64, 8 + ft, tok],
                            start=True, stop=True,
                        )
                    nc.scalar.activation(
                        e_sb[:, 4 * q4:4 * q4 + 4, :], sps[:],
                        mybir.ActivationFunctionType.Exp,
                        scale=float(EXP_SCALE),
                    )
                # causal mask for all 16 slots in one gpsimd op:
                # iota = q - k, keep where >= 0, else 0
                nc.gpsimd.affine_select(
                    out=e_sb[:], in_=e_sb[:],
                    compare_op=mybir.AluOpType.is_ge,
                    fill=0.0, base=0,
                    pattern=[[0, H], [-1, BS]],
                    channel_multiplier=1,
                )
                # den[q, slot] = sum_k e, then p = e / den (broadcast divide)
                den = den_pool.tile([128, H], F32, tag="den")
                nc.vector.tensor_reduce(
                    den[:], e_sb[:], axis=mybir.AxisListType.X,
                    op=mybir.AluOpType.add,
                )
                nc.vector.reciprocal(den[:], den[:])
                p_sb = p_pool.tile([128, H, BS], BF16, tag="p")
                den3 = den[:].rearrange("p (h o) -> p h o", o=1)
                nc.vector.tensor_tensor(
                    out=p_sb[:], in0=e_sb[:],
                    in1=_bcast_last(den3, p_sb[:]),
                    op=mybir.AluOpType.mult,
                )

                # p^T via PE transpose, 4 heads per PSUM bank -> one copy
                atn = at_pool.tile([128, KT, BS], BF16, tag="at")
                for q4 in range(4):
                    ptp = pt_ps.tile([128, 4, BS], BF16, tag="ptp")
                    for hh in range(4):
                        h = 4 * q4 + hh
                        nc.tensor.transpose(
                            ptp[:, hh, :], p_sb[:, SLOT_OF_HEAD[h], :],
                            ident[:],
                        )
                    pt = pt_pool.tile([128, 4, BS], BF16, tag="pt")
                    nc.vector.tensor_copy(pt[:], ptp[:])
                    # attn^T = v_h^T p^T, 2 heads per bank column-group;
                    # 2 quads (4 head-pairs) share one PSUM bank
                    if q4 % 2 == 0:
                        atp = at_ps.tile([128, 4, BS], F32, tag="atp")
                    for hh in range(4):
                        h = 4 * q4 + hh
                        po = 64 * (h % 2)
                        nc.tensor.matmul(
                            atp[po:po + 64, 2 * (q4 % 2) + hh // 2, :],
                            lhsT=v[:, b, h * D:(h + 1) * D],
                            rhs=pt[:, hh, :],
                            start=True, stop=True,
                            tile_position=(0, po),
                        )
                    if q4 % 2 == 1:
                        nc.scalar.activation(
                            atn[:, 2 * (q4 - 1):2 * (q4 - 1) + 4, :],
                            atp[:],
                            mybir.ActivationFunctionType.Copy,
                            scale=float(AT_SCALE),
                        )

                # proj: out[tok, cout] = attn^T.T @ W_proj + b_proj
                for ns in range(2):
                    pps = mm_ps.tile([128, 512], F32, tag="mm512")
                    for ct in range(KT):
                        nc.tensor.matmul(
                            pps[:],
                            lhsT=atn[:, ct, :],
                            rhs=wp_sb[:, ct, ns * 512:(ns + 1) * 512],
                            start=(ct == 0), stop=False,
                        )
                    nc.tensor.matmul(
                        pps[:],
                        lhsT=ones_sb[:1, :],
                        rhs=bias_sb[:1, ns * 512:(ns + 1) * 512],
                        start=False, stop=True,
                    )
                    if ns == 0:
                        nc.scalar.copy(ob[:, bj, 0:512], pps[:])
                    else:
                        nc.vector.tensor_copy(ob[:, bj, 512:1024], pps[:])

            # one store per block pair on the SP HW queue (8 total)
            r0 = t0 + bp * 2 * BS
            nc.sync.dma_start(
                out[r0:r0 + 2 * BS, :].rearrange("(blk p) c -> p blk c", p=128),
                ob[:],
            )


def _build():
    nc = bacc.Bacc()
    xhi = nc.dram_tensor("xhi", [C, TOK], F8, kind="ExternalInput")
    xlo = nc.dram_tensor("xlo", [C, TOK], F8, kind="ExternalInput")
    wqk = nc.dram_tensor("wqk", [C, 2 * C], F8, kind="ExternalInput")
    wvh = nc.dram_tensor("wvh", [C, C], F8, kind="ExternalInput")
    wvl = nc.dram_tensor("wvl", [C, C], F8, kind="ExternalInput")
    wp = nc.dram_tensor("wp", [C, C], BF16, kind="ExternalInput")
    bias = nc.dram_tensor("bias", [1, C], BF16, kind="ExternalInput")
    out = nc.dram_tensor("out", [TOK, C], F32, kind="ExternalOutput")
    with tile.TileContext(nc) as tc:
        with ExitStack() as ctx:
            _build_body(nc, tc, ctx, xhi, xlo, wqk, wvh, wvl, wp, bias, out)
    nc.finalize()
    return nc


def get_nc():
    if "nc" not in _CACHE:
        _CACHE["nc"] = _build()
    return _CACHE["nc"]


def make_in_maps(x, W_qkv, W_proj, b_proj):
    f8 = ml_dtypes.float8_e4m3
    bf = ml_dtypes.bfloat16
    x = np.asarray(x, np.float32)
    wq_s = np.asarray(W_qkv, np.float32) * W_SCALE
    wqk8 = np.ascontiguousarray(wq_s[:, :2 * C].astype(f8))
    wv_s = wq_s[:, 2 * C:]
    wvh8 = np.ascontiguousarray(wv_s.astype(f8))
    wvl8 = np.ascontiguousarray((wv_s - wvh8.astype(np.float32)).astype(f8))
    wp16 = np.ascontiguousarray(np.asarray(W_proj).astype(bf))
    bp16 = np.ascontiguousarray(np.asarray(b_proj).reshape(1, C).astype(bf))
    in_maps = []
    for s in range(N_CORES):
        bi, half = divmod(s, 2)
        xsT = np.ascontiguousarray(x[bi, half * TOK:(half + 1) * TOK].T)
        xhi = xsT.astype(f8)
        xlo = (xsT - xhi.astype(np.float32)).astype(f8)
        in_maps.append({
            "xhi": xhi, "xlo": xlo,
            "wqk": wqk8, "wvh": wvh8, "wvl": wvl8,
            "wp": wp16, "bias": bp16,
        })
    return in_maps


def kernel(x, W_qkv, W_proj, b_proj, _trace=False):
    nc = get_nc()
    in_maps = make_in_maps(x, W_qkv, W_proj, b_proj)
    res = bass_utils.run_bass_kernel_spmd(
        nc, in_maps, core_ids=list(range(N_CORES)), trace=_trace,
    )
    _CACHE["last_result"] = res
    out = np.empty((B, T, C), np.float32)
    for s in range(N_CORES):
        bi, half = divmod(s, 2)
        out[bi, half * TOK:(half + 1) * TOK] = res.results[s]["out"]
    return out
